# revision 38
# baseline (speedup 1.0000x reference)
"""CoNystromAttention Trainium2 kernel (v7).

Shard: 8 cores = 4 batches x 2 head-groups (8 heads each). Per core:
one batch b, 8 heads organized as 4 "pairs" (2 heads = 128 partitions).

Math (reference, with Q=K=V=QKV):
  QKV = X[b].T @ Wq[h].T + bq[h]                       [n=4096, d=64]
  Qt  = window-mean(QKV, 64)                           [m=64, d]
  S   = exp(QKV @ Qt.T / 8)     (Beta; Delta = S.T)    [n, m]
  G   = exp(Qt @ Qt.T / 8)
  GD  = G / rowsum(G);  V6 = newton_schulz(GD, 6)      (pinv)
  out = diag(1/r) S V6 diag(1/c) S.T QKV,  r=rowsum(S), c=colsum(S)

Design notes:
- NS init scale: rowsum(GD) == 1, so scale = 1/max(colsum) PER HEAD
  (1.3e-3 vs the reference's global max -> no collective needed).
- bf16 streams (X/Wq in, qkvt/st/qn/sn/W, out) with the NS-feeding
  landmark/Gamma path in f32; measured ~8e-3 total vs the 2e-2 gate.
- r (Beta rowsums) fall out of the final matmul via ones columns
  appended to W (cols 128/129 of the 130-wide rhs).
- NS scale folding: KV_{it+1} = 0.25*KV_it@a5 and (KV_{it+1})^T =
  0.25*a5^T@(KV_it)^T come straight from the previous iteration's
  tiles, so the V-update (psv matmul + vn scale) never sits on the
  serial chain; only the vt recurrence (for V6^T) is kept.
- M (S^T QKV) accumulates across all 32 token chunks into one PSUM
  bank; start/stop only on the very first/last matmul of the bank.
"""

import numpy as np

P = 128
N_TOK = 4096
EMBED = 1024
NPAIR = 4            # head-pairs per core (8 heads)
ECH = EMBED // P     # 8 contraction chunks
XCH = 512            # projection chunk (tokens)
NCHP = N_TOK // XCH  # 8 projection chunks
TCH = N_TOK // P     # 32 token chunks of 128
NS_ITERS = 6

_CACHE = {}


def _build(global_scale=False, debug=False):
    del global_scale  # kept for test.py compat; no collective needed
    del debug
    import concourse.mybir as mybir
    from concourse import bacc
    from concourse.tile import TileContext
    from concourse.masks import make_identity

    f32 = mybir.dt.float32
    f32r = mybir.dt.float32r
    bf16 = mybir.dt.bfloat16
    ALU = mybir.AluOpType
    ACTF = mybir.ActivationFunctionType
    AX = mybir.AxisListType

    nc = bacc.Bacc("TRN2", target_bir_lowering=False, debug=False)
    X = nc.dram_tensor("X", [EMBED, N_TOK], bf16, kind="ExternalInput")
    WqT = nc.dram_tensor("WqT", [EMBED, 512], bf16, kind="ExternalInput")
    bias = nc.dram_tensor("bias", [512], f32, kind="ExternalInput")
    out_d = nc.dram_tensor("out", [N_TOK, 512], bf16, kind="ExternalOutput")

    with TileContext(nc) as tc, (
        tc.tile_pool(name="pers", bufs=1)
    ) as pers, tc.tile_pool(name="big", bufs=1) as big:
        # ---------------- persistent tiles ----------------
        ident32 = pers.tile([P, P], f32, tag="ident32")
        make_identity(nc, ident32[:])
        identb = pers.tile([P, P], bf16, tag="identb")
        nc.vector.tensor_copy(identb[:], ident32[:])
        identr = pers.tile([P, P], f32r, tag="identr")
        nc.vector.tensor_copy(identr[:], ident32[:])
        # packed a*I constants for 4-pair NS elementwise
        i7 = pers.tile([P, 512], f32, tag="i7")
        i15 = pers.tile([P, 512], f32, tag="i15")
        i13 = pers.tile([P, 512], f32, tag="i13")
        for t, v in ((i7, 7.0), (i15, 15.0), (i13, 13.0)):
            for hh in range(4):
                nc.vector.tensor_scalar_mul(t[:, hh * P:(hh + 1) * P], ident32[:], v)
        ones_col = pers.tile([P, 1], f32r, tag="ones_col")
        nc.vector.memset(ones_col[:].bitcast(f32), 1.0)
        ones_row = pers.tile([1, P], f32r, tag="ones_row")
        nc.vector.memset(ones_row[:].bitcast(f32), 1.0)
        bias_t = pers.tile([P, NPAIR], f32, tag="bias")
        nc.sync.dma_start(bias_t[:], bias.rearrange("(f p) -> p f", p=P))
        qsum = [pers.tile([P, 64], f32, tag=f"qsum{p}", name=f"qsum{p}")
                for p in range(NPAIR)]
        cp = [pers.tile([P, NCHP], f32, tag=f"cp{p}", name=f"cp{p}")
              for p in range(NPAIR)]
        qkvt = big.tile([P, NPAIR, N_TOK], bf16, tag="qkvt")
        st = big.tile([P, NPAIR, N_TOK], bf16, tag="st")
        qn = big.tile([P, TCH, 512], bf16, tag="qn")
        wqtr = pers.tile([P, ECH, 512], bf16, tag="wqtr")
        wqre = WqT.rearrange("(eo p) hd -> p eo hd", p=P)
        nc.sync.dma_start(wqtr[:, 0:4, :], wqre[:, 0:4, :])
        nc.sync.dma_start(wqtr[:, 4:8, :], wqre[:, 4:8, :])

        # ---------------- phase B: projection ----------------
        with (
            tc.tile_pool(name="x", bufs=2) as xpool,
            tc.tile_pool(name="pp", bufs=4, space="PSUM") as pp,
            tc.tile_pool(name="tq", bufs=2, space="PSUM") as tq,
        ):
            # PE warmup while the first DMAs land: keeps the p-state ramp hot
            wzero = pers.tile([P, 512], bf16, tag="wzero")
            nc.vector.memset(wzero[:], 0.0)
            for w in range(30):
                pw = pp.tile([P, 512], f32, tag="proj", name=f"warm{w}")
                nc.tensor.matmul(pw[:], identb[:], wzero[:], start=True, stop=True)

            xre = X.rearrange("(eo p) n -> p eo n", p=P)

            def qn_transpose(c):
                # qn: transpose QKV^T chunks -> [tok, hd]
                for sc in range(4):
                    t0 = c * 4 + sc
                    tsl = slice(t0 * P, (t0 + 1) * P)
                    psQ = tq.tile([P, 512], bf16, tag="psQ", name=f"psQ{t0}")
                    for p in range(NPAIR):
                        nc.tensor.matmul(
                            psQ[:, p * P:(p + 1) * P], qkvt[:, p, tsl], identb[:],
                            is_transpose=True, start=(p == 0), stop=(p == NPAIR - 1),
                            skip_group_check=True,
                        )
                    nc.vector.tensor_copy(qn[:, t0, :], psQ[:])

            for c in range(NCHP):
                xt = xpool.tile([P, ECH, XCH], bf16, tag="xt", name=f"xt{c}")
                nc.sync.dma_start(xt[:, 0:4, :], xre[:, 0:4, c * XCH:(c + 1) * XCH])
                nc.sync.dma_start(xt[:, 4:8, :], xre[:, 4:8, c * XCH:(c + 1) * XCH])
                csl = slice(c * XCH, (c + 1) * XCH)
                for p in range(NPAIR):
                    ps = pp.tile([P, XCH], f32, tag="proj", name=f"proj{c}_{p}")
                    for e in range(ECH):
                        nc.tensor.matmul(
                            ps[:],
                            wqtr[:, e, p * P:(p + 1) * P],
                            xt[:, e, :],
                            start=(e == 0),
                            stop=(e == ECH - 1),
                        )
                    # QKV^T (bf16) = psum + bias (per-partition)
                    nc.scalar.activation(
                        qkvt[:, p, csl], ps[:], ACTF.Identity,
                        bias=bias_t[:, p:p + 1],
                    )
                    # landmark window sums (64-token windows, pre-bias f32)
                    nc.vector.reduce_sum(
                        qsum[p][:, c * 8:(c + 1) * 8],
                        ps[:].rearrange("p (w t) -> p w t", t=64),
                        axis=AX.X,
                    )
                # transposes lag one chunk so PE never waits on Act's qkvt
                if c > 0:
                    qn_transpose(c - 1)
            qn_transpose(NCHP - 1)

        # ---------------- phases C-F ----------------
        with (
            tc.tile_pool(name="wk", bufs=4) as wk,
            tc.tile_pool(name="nsv", bufs=1) as nsp,
            tc.tile_pool(name="sn", bufs=4) as snp,
            tc.tile_pool(name="ot", bufs=6) as otp,
            tc.tile_pool(name="mps", bufs=1, space="PSUM") as mps,
        ):
            sctx = tc.tile_pool(name="stps", bufs=2, space="PSUM")
            stps = sctx.__enter__()
            cctx = tc.tile_pool(name="cps", bufs=1, space="PSUM")
            cps = cctx.__enter__()

            def st_chunk(j):
                jsl = slice(j * XCH, (j + 1) * XCH)
                for p in range(NPAIR):
                    psT = stps.tile([P, 512], f32, tag="psT", name=f"psT{j}_{p}")
                    nc.tensor.matmul(
                        psT[:], blkqb[p][:], qkvt[:, p, jsl], start=True, stop=True)
                    nc.scalar.activation(
                        st[:, p, jsl], psT[:], ACTF.Exp, scale=0.125,
                        accum_out=cp[p][:, j:j + 1],
                    )

            # ---- C: landmarks, Gamma, GD, NS init (per-head scale) ----
            blkq = []
            blkqb = []
            gd = [nsp.tile([P, P], f32r, tag=f"gd{p}", name=f"gd{p}")
                  for p in range(NPAIR)]
            kt0 = nsp.tile([P, 512], bf16, tag="kt", name="kt0")
            v00 = nsp.tile([P, 512], bf16, tag="v0", name="v00")
            vt00 = nsp.tile([P, 512], bf16, tag="vt", name="vt00")

            def c_chain(p):
                hsl = slice(p * P, (p + 1) * P)
                bq_t = pers.tile([P, P], f32r, tag=f"blkq{p}", name=f"blkq{p}")
                nc.vector.memset(bq_t[:].bitcast(f32), 0.0)
                nc.vector.tensor_scalar(
                    bq_t[0:64, 0:64], qsum[p][0:64, :], 1.0 / 64,
                    bias_t[0:64, p:p + 1], ALU.mult, ALU.add,
                )
                nc.vector.tensor_scalar(
                    bq_t[64:128, 64:128], qsum[p][64:128, :], 1.0 / 64,
                    bias_t[64:128, p:p + 1], ALU.mult, ALU.add,
                )
                blkq.append(bq_t)
                bq_b = pers.tile([P, P], bf16, tag=f"blkqb{p}", name=f"blkqb{p}")
                nc.gpsimd.tensor_copy(bq_b[:], bq_t[:])
                blkqb.append(bq_b)
                psG = cps.tile([P, P], f32, tag="psG", name=f"psG{p}")
                nc.tensor.matmul(psG[:], bq_t[:], bq_t[:], start=True, stop=True)
                gm = wk.tile([P, P], f32, tag="g", name=f"g{p}")
                nc.scalar.activation(gm[:], psG[:], ACTF.Exp, scale=0.125)
                nc.gpsimd.memset(gm[0:64, 64:128], 0.0)
                nc.gpsimd.memset(gm[64:128, 0:64], 0.0)
                gs = wk.tile([P, 1], f32, tag="gs", name=f"gs{p}")
                nc.vector.reduce_sum(gs[:], gm[:], axis=AX.X)
                gri = wk.tile([P, 1], f32, tag="gri", name=f"gri{p}")
                nc.vector.reciprocal(gri[:], gs[:])
                nc.vector.tensor_scalar_mul(gd[p][:], gm[:], gri[:])
                # per-head scale: rowsum(GD)=1 -> s = 1/max(colsum(GD)) per head
                psc = cps.tile([P, P], f32, tag="psc", name=f"psc{p}")
                nc.tensor.matmul(
                    psc[0:1, :], ones_col[:], gd[p][:], start=True, stop=True)
                cm = wk.tile([1, 2], f32, tag="cm", name=f"cm{p}")
                nc.vector.reduce_max(
                    cm[:], psc[0:1, :].rearrange("p (h l) -> p h l", l=64), axis=AX.X)
                cmi = wk.tile([1, 2], f32, tag="cmi", name=f"cmi{p}")
                nc.vector.reciprocal(cmi[:], cm[:])
                cmib = wk.tile([1, 2], f32r, tag="cmib", name=f"cmib{p}")
                nc.gpsimd.tensor_copy(cmib[:], cmi[:])
                psb = cps.tile([P, 2], f32, tag="psb", name=f"psb{p}")
                nc.tensor.matmul(psb[:], ones_row[:], cmib[:], start=True, stop=True)
                sv = wk.tile([P, 2], f32, tag="sv", name=f"sv{p}")
                nc.vector.tensor_copy(sv[:], psb[:])
                pskt = cps.tile([P, P], f32r, tag="pskt", name=f"pskt{p}")
                nc.tensor.matmul(pskt[:], gd[p][:], identr[:], is_transpose=True)
                nc.vector.tensor_copy(kt0[:, hsl], pskt[:])
                # v0 = s * K^T, vt0 = s * K  (s per head = per row-half)
                nc.vector.tensor_scalar_mul(
                    v00[0:64, hsl], pskt[0:64, :], sv[0:64, 0:1])
                nc.vector.tensor_scalar_mul(
                    v00[64:128, hsl], pskt[64:128, :], sv[64:128, 1:2])
                nc.gpsimd.tensor_scalar_mul(
                    vt00[0:64, hsl], gd[p][0:64, :], sv[0:64, 0:1])
                nc.gpsimd.tensor_scalar_mul(
                    vt00[64:128, hsl], gd[p][64:128, :], sv[64:128, 1:2])

            c_chain(0)
            c_chain(1)
            c_chain(2)
            c_chain(3)
            st_chunk(0)
            st_chunk(1)
            cctx.__exit__(None, None, None)
            mbank = mps.tile([P, NPAIR, P], f32, tag="mbank")

            # ---- D: interleaved NS iterations + ST + token loop ----
            dctx = tc.tile_pool(name="nsps", bufs=3, space="PSUM")
            nsps = dctx.__enter__()
            dctx3 = tc.tile_pool(name="tp2", bufs=2, space="PSUM")
            tp2 = dctx3.__enter__()
            quarts = [slice(h * P, (h + 1) * P) for h in range(4)]

            def ns_mm4(name, lhs, rhs):
                ps = nsps.tile([P, 512], f32, tag="nsb", name=name)
                for h, hs in enumerate(quarts):
                    nc.tensor.matmul(ps[:, hs], lhs[:, hs], rhs[:, hs],
                                     start=(h == 0), stop=(h == 3),
                                     skip_group_check=True)
                return ps

            ns_state = {"v": v00, "vt": vt00}

            def ns_step(it):
                v, vt = ns_state["v"], ns_state["vt"]
                pskv = ns_mm4(f"pskv{it}", kt0, v)
                pskvt = ns_mm4(f"pskvt{it}", v, kt0)
                a1 = nsp.tile([P, 512], bf16, tag="a1", name=f"a1_{it}")
                nc.vector.tensor_tensor(a1[:], i7[:], pskv[:], ALU.subtract)
                kvt = nsp.tile([P, 512], bf16, tag="kvt", name=f"kvt_{it}")
                nc.scalar.copy(kvt[:], pskvt[:])
                psa2 = ns_mm4(f"psa2_{it}", kvt, a1)
                a3 = nsp.tile([P, 512], bf16, tag="a3", name=f"a3_{it}")
                nc.vector.tensor_tensor(a3[:], i15[:], psa2[:], ALU.subtract)
                psa4 = ns_mm4(f"psa4_{it}", kvt, a3)
                a5 = nsp.tile([P, 512], bf16, tag="a5", name=f"a5_{it}")
                nc.vector.tensor_tensor(a5[:], i13[:], psa4[:], ALU.subtract)
                if it < NS_ITERS - 1:
                    psv = ns_mm4(f"psv_{it}", vt, a5)
                    vn = nsp.tile([P, 512], bf16, tag="v0", name=f"vn_{it}")
                    nc.vector.tensor_scalar_mul(vn[:], psv[:], 0.25)
                else:
                    vn = v
                psvt2 = ns_mm4(f"psvt2_{it}", a5, vt)
                vtn = nsp.tile([P, 512], bf16, tag="vt", name=f"vtn_{it}")
                nc.vector.tensor_scalar_mul(vtn[:], psvt2[:], 0.25)
                ns_state.update(v=vn, vt=vtn)

            ns_per_j = [1, 1, 1, 1, 1, 1, 0, 0]
            ns_i = 0

            for j in range(NCHP):
                for _ in range(ns_per_j[j]):
                    ns_step(ns_i)
                    ns_i += 1
                if j >= 2:
                    st_chunk(j)
                for sc in range(4):
                    t0 = j * 4 + sc
                    tsl = slice(t0 * P, (t0 + 1) * P)
                    psS = tp2.tile([P, 512], bf16, tag="psS", name=f"psS{t0}")
                    for p in range(NPAIR):
                        nc.tensor.matmul(
                            psS[:, p * P:(p + 1) * P], st[:, p, tsl], identb[:],
                            is_transpose=True, start=(p == 0), stop=(p == NPAIR - 1),
                            skip_group_check=True,
                        )
                    sn = snp.tile([P, 512], bf16, tag="sn", name=f"sn{t0}")
                    nc.vector.tensor_copy(sn[:], psS[:])
                    for p in range(NPAIR):
                        nc.tensor.matmul(
                            mbank[:, p, :], sn[:, p * P:(p + 1) * P],
                            qn[:, t0, p * P:(p + 1) * P],
                            start=(t0 == 0 and p == 0),
                            stop=(t0 == TCH - 1 and p == NPAIR - 1),
                            skip_group_check=True,
                        )

            dctx3.__exit__(None, None, None)
            dctx.__exit__(None, None, None)
            sctx.__exit__(None, None, None)

            # ---- E: W = V6 @ (diag(1/c) M), plus ones cols for r ----
            ectx = tc.tile_pool(name="wps", bufs=1, space="PSUM")
            wps = ectx.__enter__()
            wpads = []
            for p in range(NPAIR):
                cs = wk.tile([P, 1], f32, tag="cs", name=f"cs{p}")
                nc.vector.reduce_sum(cs[:], cp[p][:], axis=AX.X)
                cinv = wk.tile([P, 1], f32, tag="cinv", name=f"cinv{p}")
                nc.vector.reciprocal(cinv[:], cs[:])
                dvp = wk.tile([P, P], bf16, tag="dvp", name=f"dvp{p}")
                nc.vector.tensor_scalar_mul(dvp[:], mbank[:, p, :], cinv[:])
                # zero cross-head blocks (S/V are dense across the pair)
                nc.vector.memset(dvp[0:64, 64:128], 0.0)
                nc.vector.memset(dvp[64:128, 0:64], 0.0)
                psw = wps.tile([P, P], f32, tag="psw", name=f"psw{p}")
                vt6 = ns_state["vt"]
                nc.tensor.matmul(psw[:], vt6[:, p * P:(p + 1) * P], dvp[:],
                                 start=True, stop=True)
                wpad = pers.tile([P, 132], bf16, tag=f"wpad{p}")
                nc.vector.memset(wpad[:], 0.0)
                nc.scalar.copy(wpad[:, 0:P], psw[:])
                nc.vector.memset(wpad[0:64, 128:129], 1.0)
                nc.vector.memset(wpad[64:128, 129:130], 1.0)
                wpads.append(wpad)

            # ---- F: out = diag(1/r) S W ----
            fctx = tc.tile_pool(name="fps", bufs=6, space="PSUM")
            fps = fctx.__enter__()
            unit = 0
            for t in range(TCH):
                tsl = slice(t * P, (t + 1) * P)
                if t % 2 == 0:
                    ot4 = otp.tile([P, 2, 512], bf16, tag="ot", name=f"ot{t}")
                ot = ot4[:, t % 2, :]
                for q in range(2):
                    pso = fps.tile([P, 260], f32, tag="pso", name=f"pso{q}_{t}")
                    for jj in range(2):
                        p = 2 * q + jj
                        nc.tensor.matmul(
                            pso[:, jj * 130:(jj + 1) * 130], st[:, p, tsl],
                            wpads[p][:, 0:130],
                            start=(jj == 0), stop=(jj == 1), skip_group_check=True,
                        )
                    rv4 = wk.tile([P, 2, 2, 1], f32, tag=f"rv{q}", name=f"rv{q}_{t}")
                    osl = ot[:, q * 256:(q + 1) * 256].rearrange(
                        "p (b h d) -> p b h d", h=2, d=64)
                    if unit % 2 == 0:
                        data = pso[:].rearrange("p (b x) -> p b x", b=2)
                        nc.vector.reciprocal(rv4[:, :, :, 0], data[:, :, 128:130])
                        nc.vector.tensor_tensor(
                            osl,
                            data[:, :, 0:128].rearrange("p b (h d) -> p b h d", d=64),
                            rv4[:].to_broadcast([P, 2, 2, 64]),
                            ALU.mult,
                        )
                    else:
                        pf = snp.tile([P, 260], f32, tag="pf", name=f"pf{t}_{q}")
                        nc.scalar.copy(pf[:], pso[:])
                        dataf = pf[:].rearrange("p (b x) -> p b x", b=2)
                        nc.vector.reciprocal(rv4[:, :, :, 0], dataf[:, :, 128:130])
                        nc.gpsimd.tensor_tensor(
                            osl,
                            dataf[:, :, 0:128].rearrange("p b (h d) -> p b h d", d=64),
                            rv4[:].to_broadcast([P, 2, 2, 64]),
                            ALU.mult,
                        )
                    unit += 1
                if t % 2 == 1:
                    t4sl = slice((t - 1) * P, (t + 1) * P)
                    nc.sync.dma_start(
                        out_d[t4sl, :].rearrange("(c p) n -> p c n", p=P), ot4[:])
            fctx.__exit__(None, None, None)
            ectx.__exit__(None, None, None)

    nc.compile()
    return nc


def _get_nc():
    if "nc" not in _CACHE:
        _CACHE["nc"] = _build()
    return _CACHE["nc"]


def kernel(X, Wq, bq):
    import ml_dtypes
    from concourse.bass_utils import run_bass_kernel_spmd

    nc = _get_nc()
    B, E, n = X.shape
    H = Wq.shape[0]
    in_maps = []
    for core in range(8):
        b = core // 2
        h0 = 8 * (core % 2)
        wq_c = Wq[h0:h0 + 8]                      # [8, 64, 1024]
        wqt_c = np.ascontiguousarray(
            wq_c.transpose(2, 0, 1).reshape(E, 512)).astype(ml_dtypes.bfloat16)
        bias_c = np.ascontiguousarray(bq[h0:h0 + 8].reshape(512))
        in_maps.append({
            "X": np.ascontiguousarray(X[b]).astype(ml_dtypes.bfloat16),
            "WqT": wqt_c,
            "bias": bias_c,
        })
    res = run_bass_kernel_spmd(nc, in_maps, core_ids=list(range(8)))
    out = np.empty((B, H, n, 64), dtype=np.float32)
    for core in range(8):
        b = core // 2
        h0 = 8 * (core % 2)
        oc = np.asarray(res.results[core]["out"]).astype(np.float32).reshape(n, 8, 64)
        out[b, h0:h0 + 8] = oc.transpose(1, 0, 2)
    return out


# revision 43
# speedup vs baseline: 1.0156x; 1.0156x over previous
"""CoNystromAttention Trainium2 kernel (v7).

Shard: 8 cores = 4 batches x 2 head-groups (8 heads each). Per core:
one batch b, 8 heads organized as 4 "pairs" (2 heads = 128 partitions).

Math (reference, with Q=K=V=QKV):
  QKV = X[b].T @ Wq[h].T + bq[h]                       [n=4096, d=64]
  Qt  = window-mean(QKV, 64)                           [m=64, d]
  S   = exp(QKV @ Qt.T / 8)     (Beta; Delta = S.T)    [n, m]
  G   = exp(Qt @ Qt.T / 8)
  GD  = G / rowsum(G);  V6 = newton_schulz(GD, 6)      (pinv)
  out = diag(1/r) S V6 diag(1/c) S.T QKV,  r=rowsum(S), c=colsum(S)

Design notes:
- NS init scale: rowsum(GD) == 1, so scale = 1/max(colsum) PER HEAD
  (1.3e-3 vs the reference's global max -> no collective needed).
- bf16 streams (X/Wq in, qkvt/st/qn/sn/W, out) with the NS-feeding
  landmark/Gamma path in f32; measured ~8e-3 total vs the 2e-2 gate.
- r (Beta rowsums) fall out of the final matmul via ones columns
  appended to W (cols 128/129 of the 130-wide rhs).
- NS scale folding: KV_{it+1} = 0.25*KV_it@a5 and (KV_{it+1})^T =
  0.25*a5^T@(KV_it)^T come straight from the previous iteration's
  tiles, so the V-update (psv matmul + vn scale) never sits on the
  serial chain; only the vt recurrence (for V6^T) is kept.
- M (S^T QKV) accumulates across all 32 token chunks into one PSUM
  bank; start/stop only on the very first/last matmul of the bank.
"""

import numpy as np

P = 128
N_TOK = 4096
EMBED = 1024
NPAIR = 4            # head-pairs per core (8 heads)
ECH = EMBED // P     # 8 contraction chunks
XCH = 512            # projection chunk (tokens)
NCHP = N_TOK // XCH  # 8 projection chunks
TCH = N_TOK // P     # 32 token chunks of 128
NS_ITERS = 6

_CACHE = {}


def _build(global_scale=False, debug=False):
    del global_scale  # kept for test.py compat; no collective needed
    del debug
    import concourse.mybir as mybir
    from concourse import bacc
    from concourse.tile import TileContext
    from concourse.masks import make_identity

    f32 = mybir.dt.float32
    f32r = mybir.dt.float32r
    bf16 = mybir.dt.bfloat16
    ALU = mybir.AluOpType
    ACTF = mybir.ActivationFunctionType
    AX = mybir.AxisListType

    nc = bacc.Bacc("TRN2", target_bir_lowering=False, debug=False)
    X = nc.dram_tensor("X", [EMBED, N_TOK], bf16, kind="ExternalInput")
    WqT = nc.dram_tensor("WqT", [EMBED, 512], bf16, kind="ExternalInput")
    bias = nc.dram_tensor("bias", [512], f32, kind="ExternalInput")
    out_d = nc.dram_tensor("out", [N_TOK, 512], bf16, kind="ExternalOutput")

    with TileContext(nc) as tc, (
        tc.tile_pool(name="pers", bufs=1)
    ) as pers, tc.tile_pool(name="big", bufs=1) as big:
        # ---------------- persistent tiles ----------------
        ident32 = pers.tile([P, P], f32, tag="ident32")
        make_identity(nc, ident32[:])
        identb = pers.tile([P, P], bf16, tag="identb")
        nc.vector.tensor_copy(identb[:], ident32[:])
        identr = pers.tile([P, P], f32r, tag="identr")
        nc.vector.tensor_copy(identr[:], ident32[:])
        # packed a*I constants for 4-pair NS elementwise
        i7 = pers.tile([P, 512], f32, tag="i7")
        i15 = pers.tile([P, 512], f32, tag="i15")
        i13 = pers.tile([P, 512], f32, tag="i13")
        for t, v in ((i7, 7.0), (i15, 15.0), (i13, 13.0)):
            for hh in range(4):
                nc.vector.tensor_scalar_mul(t[:, hh * P:(hh + 1) * P], ident32[:], v)
        ones_col = pers.tile([P, 1], f32r, tag="ones_col")
        nc.vector.memset(ones_col[:].bitcast(f32), 1.0)
        ones_row = pers.tile([1, P], f32r, tag="ones_row")
        nc.vector.memset(ones_row[:].bitcast(f32), 1.0)
        bias_t = pers.tile([P, NPAIR], f32, tag="bias")
        nc.sync.dma_start(bias_t[:], bias.rearrange("(f p) -> p f", p=P))
        qsum = [pers.tile([P, 64], f32, tag=f"qsum{p}", name=f"qsum{p}")
                for p in range(NPAIR)]
        cp = [pers.tile([P, NCHP], f32, tag=f"cp{p}", name=f"cp{p}")
              for p in range(NPAIR)]
        qkvt = big.tile([P, NPAIR, N_TOK], bf16, tag="qkvt")
        st = big.tile([P, NPAIR, N_TOK], bf16, tag="st")
        qn = big.tile([P, TCH, 512], bf16, tag="qn")
        wqtr = pers.tile([P, ECH, 512], bf16, tag="wqtr")
        wqre = WqT.rearrange("(eo p) hd -> p eo hd", p=P)
        nc.sync.dma_start(wqtr[:, 0:4, :], wqre[:, 0:4, :])
        nc.sync.dma_start(wqtr[:, 4:8, :], wqre[:, 4:8, :])

        # ---------------- phase B: projection ----------------
        with (
            tc.tile_pool(name="x", bufs=2) as xpool,
            tc.tile_pool(name="pp", bufs=4, space="PSUM") as pp,
            tc.tile_pool(name="tq", bufs=2, space="PSUM") as tq,
        ):
            # PE warmup while the first DMAs land: keeps the p-state ramp hot
            wzero = pers.tile([P, 512], bf16, tag="wzero")
            nc.vector.memset(wzero[:], 0.0)
            for w in range(14):
                pw = pp.tile([P, 512], f32, tag="proj", name=f"warm{w}")
                nc.tensor.matmul(pw[:], identb[:], wzero[:], start=True, stop=True)

            xre = X.rearrange("(eo p) n -> p eo n", p=P)

            def qn_transpose(c):
                # qn: transpose QKV^T chunks -> [tok, hd]
                for sc in range(4):
                    t0 = c * 4 + sc
                    tsl = slice(t0 * P, (t0 + 1) * P)
                    psQ = tq.tile([P, 512], bf16, tag="psQ", name=f"psQ{t0}")
                    for p in range(NPAIR):
                        nc.tensor.matmul(
                            psQ[:, p * P:(p + 1) * P], qkvt[:, p, tsl], identb[:],
                            is_transpose=True, start=(p == 0), stop=(p == NPAIR - 1),
                            skip_group_check=True,
                        )
                    nc.vector.tensor_copy(qn[:, t0, :], psQ[:])

            for c in range(NCHP):
                xt = xpool.tile([P, ECH, XCH], bf16, tag="xt", name=f"xt{c}")
                nc.sync.dma_start(xt[:, 0:4, :], xre[:, 0:4, c * XCH:(c + 1) * XCH])
                nc.sync.dma_start(xt[:, 4:8, :], xre[:, 4:8, c * XCH:(c + 1) * XCH])
                csl = slice(c * XCH, (c + 1) * XCH)
                for p in range(NPAIR):
                    ps = pp.tile([P, XCH], f32, tag="proj", name=f"proj{c}_{p}")
                    for e in range(ECH):
                        nc.tensor.matmul(
                            ps[:],
                            wqtr[:, e, p * P:(p + 1) * P],
                            xt[:, e, :],
                            start=(e == 0),
                            stop=(e == ECH - 1),
                        )
                    # QKV^T (bf16) = psum + bias (per-partition)
                    nc.scalar.activation(
                        qkvt[:, p, csl], ps[:], ACTF.Identity,
                        bias=bias_t[:, p:p + 1],
                    )
                    # landmark window sums (64-token windows, pre-bias f32)
                    nc.vector.reduce_sum(
                        qsum[p][:, c * 8:(c + 1) * 8],
                        ps[:].rearrange("p (w t) -> p w t", t=64),
                        axis=AX.X,
                    )
                # transposes lag one chunk so PE never waits on Act's qkvt
                if c > 0:
                    qn_transpose(c - 1)
            qn_transpose(NCHP - 1)

        # ---------------- phases C-F ----------------
        with (
            tc.tile_pool(name="wk", bufs=4) as wk,
            tc.tile_pool(name="nsv", bufs=1) as nsp,
            tc.tile_pool(name="sn", bufs=4) as snp,
            tc.tile_pool(name="ot", bufs=6) as otp,
            tc.tile_pool(name="mps", bufs=1, space="PSUM") as mps,
        ):
            sctx = tc.tile_pool(name="stps", bufs=2, space="PSUM")
            stps = sctx.__enter__()
            cctx = tc.tile_pool(name="cps", bufs=1, space="PSUM")
            cps = cctx.__enter__()

            def st_chunk(j):
                jsl = slice(j * XCH, (j + 1) * XCH)
                for p in range(NPAIR):
                    psT = stps.tile([P, 512], f32, tag="psT", name=f"psT{j}_{p}")
                    nc.tensor.matmul(
                        psT[:], blkqb[p][:], qkvt[:, p, jsl], start=True, stop=True)
                    nc.scalar.activation(
                        st[:, p, jsl], psT[:], ACTF.Exp, scale=0.125,
                        accum_out=cp[p][:, j:j + 1],
                    )

            # ---- C: landmarks, Gamma, GD, NS init (per-head scale) ----
            blkq = []
            blkqb = []
            gd = [nsp.tile([P, P], f32r, tag=f"gd{p}", name=f"gd{p}")
                  for p in range(NPAIR)]
            kt0 = nsp.tile([P, 512], bf16, tag="kt", name="kt0")
            v00 = nsp.tile([P, 512], bf16, tag="v0", name="v00")
            vt00 = nsp.tile([P, 512], bf16, tag="vt", name="vt00")

            def c_chain(p):
                hsl = slice(p * P, (p + 1) * P)
                bq_t = pers.tile([P, P], f32r, tag=f"blkq{p}", name=f"blkq{p}")
                nc.vector.memset(bq_t[:].bitcast(f32), 0.0)
                nc.vector.tensor_scalar(
                    bq_t[0:64, 0:64], qsum[p][0:64, :], 1.0 / 64,
                    bias_t[0:64, p:p + 1], ALU.mult, ALU.add,
                )
                nc.vector.tensor_scalar(
                    bq_t[64:128, 64:128], qsum[p][64:128, :], 1.0 / 64,
                    bias_t[64:128, p:p + 1], ALU.mult, ALU.add,
                )
                blkq.append(bq_t)
                bq_b = pers.tile([P, P], bf16, tag=f"blkqb{p}", name=f"blkqb{p}")
                nc.gpsimd.tensor_copy(bq_b[:], bq_t[:])
                blkqb.append(bq_b)
                psG = cps.tile([P, P], f32, tag="psG", name=f"psG{p}")
                nc.tensor.matmul(psG[:], bq_t[:], bq_t[:], start=True, stop=True)
                gm = wk.tile([P, P], f32, tag="g", name=f"g{p}")
                nc.scalar.activation(gm[:], psG[:], ACTF.Exp, scale=0.125)
                nc.gpsimd.memset(gm[0:64, 64:128], 0.0)
                nc.gpsimd.memset(gm[64:128, 0:64], 0.0)
                gs = wk.tile([P, 1], f32, tag="gs", name=f"gs{p}")
                nc.vector.reduce_sum(gs[:], gm[:], axis=AX.X)
                gri = wk.tile([P, 1], f32, tag="gri", name=f"gri{p}")
                nc.vector.reciprocal(gri[:], gs[:])
                nc.vector.tensor_scalar_mul(gd[p][:], gm[:], gri[:])
                # per-head scale: rowsum(GD)=1 -> s = 1/max(colsum(GD)) per head
                psc = cps.tile([P, P], f32, tag="psc", name=f"psc{p}")
                nc.tensor.matmul(
                    psc[0:1, :], ones_col[:], gd[p][:], start=True, stop=True)
                cm = wk.tile([1, 2], f32, tag="cm", name=f"cm{p}")
                nc.vector.reduce_max(
                    cm[:], psc[0:1, :].rearrange("p (h l) -> p h l", l=64), axis=AX.X)
                cmi = wk.tile([1, 2], f32, tag="cmi", name=f"cmi{p}")
                nc.vector.reciprocal(cmi[:], cm[:])
                cmib = wk.tile([1, 2], f32r, tag="cmib", name=f"cmib{p}")
                nc.gpsimd.tensor_copy(cmib[:], cmi[:])
                psb = cps.tile([P, 2], f32, tag="psb", name=f"psb{p}")
                nc.tensor.matmul(psb[:], ones_row[:], cmib[:], start=True, stop=True)
                sv = wk.tile([P, 2], f32, tag="sv", name=f"sv{p}")
                nc.vector.tensor_copy(sv[:], psb[:])
                pskt = cps.tile([P, P], f32r, tag="pskt", name=f"pskt{p}")
                nc.tensor.matmul(pskt[:], gd[p][:], identr[:], is_transpose=True)
                nc.vector.tensor_copy(kt0[:, hsl], pskt[:])
                # v0 = s * K^T, vt0 = s * K  (s per head = per row-half)
                nc.vector.tensor_scalar_mul(
                    v00[0:64, hsl], pskt[0:64, :], sv[0:64, 0:1])
                nc.vector.tensor_scalar_mul(
                    v00[64:128, hsl], pskt[64:128, :], sv[64:128, 1:2])
                nc.gpsimd.tensor_scalar_mul(
                    vt00[0:64, hsl], gd[p][0:64, :], sv[0:64, 0:1])
                nc.gpsimd.tensor_scalar_mul(
                    vt00[64:128, hsl], gd[p][64:128, :], sv[64:128, 1:2])

            c_chain(0)
            c_chain(1)
            c_chain(2)
            c_chain(3)
            st_chunk(0)
            st_chunk(1)
            cctx.__exit__(None, None, None)
            mbank = mps.tile([P, NPAIR, P], f32, tag="mbank")

            # ---- D: interleaved NS iterations + ST + token loop ----
            dctx = tc.tile_pool(name="nsps", bufs=3, space="PSUM")
            nsps = dctx.__enter__()
            dctx3 = tc.tile_pool(name="tp2", bufs=2, space="PSUM")
            tp2 = dctx3.__enter__()
            quarts = [slice(h * P, (h + 1) * P) for h in range(4)]

            def ns_mm4(name, lhs, rhs):
                ps = nsps.tile([P, 512], f32, tag="nsb", name=name)
                for h, hs in enumerate(quarts):
                    nc.tensor.matmul(ps[:, hs], lhs[:, hs], rhs[:, hs],
                                     start=(h == 0), stop=(h == 3),
                                     skip_group_check=True)
                return ps

            ns_state = {"v": v00, "vt": vt00}

            def ns_step(it):
                v, vt = ns_state["v"], ns_state["vt"]
                pskv = ns_mm4(f"pskv{it}", kt0, v)
                pskvt = ns_mm4(f"pskvt{it}", v, kt0)
                a1 = nsp.tile([P, 512], bf16, tag="a1", name=f"a1_{it}")
                nc.vector.tensor_tensor(a1[:], i7[:], pskv[:], ALU.subtract)
                kvt = nsp.tile([P, 512], bf16, tag="kvt", name=f"kvt_{it}")
                nc.scalar.copy(kvt[:], pskvt[:])
                psa2 = ns_mm4(f"psa2_{it}", kvt, a1)
                a3 = nsp.tile([P, 512], bf16, tag="a3", name=f"a3_{it}")
                nc.vector.tensor_tensor(a3[:], i15[:], psa2[:], ALU.subtract)
                psa4 = ns_mm4(f"psa4_{it}", kvt, a3)
                a5 = nsp.tile([P, 512], bf16, tag="a5", name=f"a5_{it}")
                nc.vector.tensor_tensor(a5[:], i13[:], psa4[:], ALU.subtract)
                if it < NS_ITERS - 1:
                    psv = ns_mm4(f"psv_{it}", vt, a5)
                    vn = nsp.tile([P, 512], bf16, tag="v0", name=f"vn_{it}")
                    nc.vector.tensor_scalar_mul(vn[:], psv[:], 0.25)
                else:
                    vn = v
                psvt2 = ns_mm4(f"psvt2_{it}", a5, vt)
                vtn = nsp.tile([P, 512], bf16, tag="vt", name=f"vtn_{it}")
                nc.vector.tensor_scalar_mul(vtn[:], psvt2[:], 0.25)
                ns_state.update(v=vn, vt=vtn)

            ns_per_j = [1, 1, 1, 1, 1, 1, 0, 0]
            ns_i = 0

            for j in range(NCHP):
                for _ in range(ns_per_j[j]):
                    ns_step(ns_i)
                    ns_i += 1
                if j >= 2:
                    st_chunk(j)
                for sc in range(4):
                    t0 = j * 4 + sc
                    tsl = slice(t0 * P, (t0 + 1) * P)
                    psS = tp2.tile([P, 512], bf16, tag="psS", name=f"psS{t0}")
                    for p in range(NPAIR):
                        nc.tensor.matmul(
                            psS[:, p * P:(p + 1) * P], st[:, p, tsl], identb[:],
                            is_transpose=True, start=(p == 0), stop=(p == NPAIR - 1),
                            skip_group_check=True,
                        )
                    sn = snp.tile([P, 512], bf16, tag="sn", name=f"sn{t0}")
                    nc.vector.tensor_copy(sn[:], psS[:])
                    for p in range(NPAIR):
                        nc.tensor.matmul(
                            mbank[:, p, :], sn[:, p * P:(p + 1) * P],
                            qn[:, t0, p * P:(p + 1) * P],
                            start=(t0 == 0 and p == 0),
                            stop=(t0 == TCH - 1 and p == NPAIR - 1),
                            skip_group_check=True,
                        )

            dctx3.__exit__(None, None, None)
            dctx.__exit__(None, None, None)
            sctx.__exit__(None, None, None)

            # ---- E: W = V6 @ (diag(1/c) M), plus ones cols for r ----
            ectx = tc.tile_pool(name="wps", bufs=1, space="PSUM")
            wps = ectx.__enter__()
            wpads = []
            for p in range(NPAIR):
                cs = wk.tile([P, 1], f32, tag="cs", name=f"cs{p}")
                nc.vector.reduce_sum(cs[:], cp[p][:], axis=AX.X)
                cinv = wk.tile([P, 1], f32, tag="cinv", name=f"cinv{p}")
                nc.vector.reciprocal(cinv[:], cs[:])
                dvp = wk.tile([P, P], bf16, tag="dvp", name=f"dvp{p}")
                nc.vector.tensor_scalar_mul(dvp[:], mbank[:, p, :], cinv[:])
                # zero cross-head blocks (S/V are dense across the pair)
                nc.vector.memset(dvp[0:64, 64:128], 0.0)
                nc.vector.memset(dvp[64:128, 0:64], 0.0)
                psw = wps.tile([P, P], f32, tag="psw", name=f"psw{p}")
                vt6 = ns_state["vt"]
                nc.tensor.matmul(psw[:], vt6[:, p * P:(p + 1) * P], dvp[:],
                                 start=True, stop=True)
                wpad = pers.tile([P, 132], bf16, tag=f"wpad{p}")
                nc.vector.memset(wpad[:], 0.0)
                nc.scalar.copy(wpad[:, 0:P], psw[:])
                nc.vector.memset(wpad[0:64, 128:129], 1.0)
                nc.vector.memset(wpad[64:128, 129:130], 1.0)
                wpads.append(wpad)

            # ---- F: out = diag(1/r) S W ----
            fctx = tc.tile_pool(name="fps", bufs=6, space="PSUM")
            fps = fctx.__enter__()
            unit = 0
            for t in range(TCH):
                tsl = slice(t * P, (t + 1) * P)
                if t % 2 == 0:
                    ot4 = otp.tile([P, 2, 512], bf16, tag="ot", name=f"ot{t}")
                ot = ot4[:, t % 2, :]
                for q in range(2):
                    pso = fps.tile([P, 260], f32, tag="pso", name=f"pso{q}_{t}")
                    for jj in range(2):
                        p = 2 * q + jj
                        nc.tensor.matmul(
                            pso[:, jj * 130:(jj + 1) * 130], st[:, p, tsl],
                            wpads[p][:, 0:130],
                            start=(jj == 0), stop=(jj == 1), skip_group_check=True,
                        )
                    rv4 = wk.tile([P, 2, 2, 1], f32, tag=f"rv{q}", name=f"rv{q}_{t}")
                    osl = ot[:, q * 256:(q + 1) * 256].rearrange(
                        "p (b h d) -> p b h d", h=2, d=64)
                    if unit % 2 == 0:
                        data = pso[:].rearrange("p (b x) -> p b x", b=2)
                        nc.vector.reciprocal(rv4[:, :, :, 0], data[:, :, 128:130])
                        nc.vector.tensor_tensor(
                            osl,
                            data[:, :, 0:128].rearrange("p b (h d) -> p b h d", d=64),
                            rv4[:].to_broadcast([P, 2, 2, 64]),
                            ALU.mult,
                        )
                    else:
                        pf = snp.tile([P, 260], f32, tag="pf", name=f"pf{t}_{q}")
                        nc.scalar.copy(pf[:], pso[:])
                        dataf = pf[:].rearrange("p (b x) -> p b x", b=2)
                        nc.vector.reciprocal(rv4[:, :, :, 0], dataf[:, :, 128:130])
                        nc.gpsimd.tensor_tensor(
                            osl,
                            dataf[:, :, 0:128].rearrange("p b (h d) -> p b h d", d=64),
                            rv4[:].to_broadcast([P, 2, 2, 64]),
                            ALU.mult,
                        )
                    unit += 1
                if t % 2 == 1:
                    t4sl = slice((t - 1) * P, (t + 1) * P)
                    nc.sync.dma_start(
                        out_d[t4sl, :].rearrange("(c p) n -> p c n", p=P), ot4[:])
            fctx.__exit__(None, None, None)
            ectx.__exit__(None, None, None)

    nc.compile()
    return nc


def _get_nc():
    if "nc" not in _CACHE:
        _CACHE["nc"] = _build()
    return _CACHE["nc"]


def kernel(X, Wq, bq):
    import ml_dtypes
    from concourse.bass_utils import run_bass_kernel_spmd

    nc = _get_nc()
    B, E, n = X.shape
    H = Wq.shape[0]
    in_maps = []
    for core in range(8):
        b = core // 2
        h0 = 8 * (core % 2)
        wq_c = Wq[h0:h0 + 8]                      # [8, 64, 1024]
        wqt_c = np.ascontiguousarray(
            wq_c.transpose(2, 0, 1).reshape(E, 512)).astype(ml_dtypes.bfloat16)
        bias_c = np.ascontiguousarray(bq[h0:h0 + 8].reshape(512))
        in_maps.append({
            "X": np.ascontiguousarray(X[b]).astype(ml_dtypes.bfloat16),
            "WqT": wqt_c,
            "bias": bias_c,
        })
    res = run_bass_kernel_spmd(nc, in_maps, core_ids=list(range(8)))
    out = np.empty((B, H, n, 64), dtype=np.float32)
    for core in range(8):
        b = core // 2
        h0 = 8 * (core % 2)
        oc = np.asarray(res.results[core]["out"]).astype(np.float32).reshape(n, 8, 64)
        out[b, h0:h0 + 8] = oc.transpose(1, 0, 2)
    return out


# revision 49
# speedup vs baseline: 1.0224x; 1.0067x over previous
"""CoNystromAttention Trainium2 kernel (v7).

Shard: 8 cores = 4 batches x 2 head-groups (8 heads each). Per core:
one batch b, 8 heads organized as 4 "pairs" (2 heads = 128 partitions).

Math (reference, with Q=K=V=QKV):
  QKV = X[b].T @ Wq[h].T + bq[h]                       [n=4096, d=64]
  Qt  = window-mean(QKV, 64)                           [m=64, d]
  S   = exp(QKV @ Qt.T / 8)     (Beta; Delta = S.T)    [n, m]
  G   = exp(Qt @ Qt.T / 8)
  GD  = G / rowsum(G);  V6 = newton_schulz(GD, 6)      (pinv)
  out = diag(1/r) S V6 diag(1/c) S.T QKV,  r=rowsum(S), c=colsum(S)

Design notes:
- NS init scale: rowsum(GD) == 1, so scale = 1/max(colsum) PER HEAD
  (1.3e-3 vs the reference's global max -> no collective needed).
- bf16 streams (X/Wq in, qkvt/st/qn/sn/W, out) with the NS-feeding
  landmark/Gamma path in f32; measured ~8e-3 total vs the 2e-2 gate.
- r (Beta rowsums) fall out of the final matmul via ones columns
  appended to W (cols 128/129 of the 130-wide rhs).
- NS scale folding: KV_{it+1} = 0.25*KV_it@a5 and (KV_{it+1})^T =
  0.25*a5^T@(KV_it)^T come straight from the previous iteration's
  tiles, so the V-update (psv matmul + vn scale) never sits on the
  serial chain; only the vt recurrence (for V6^T) is kept.
- M (S^T QKV) accumulates across all 32 token chunks into one PSUM
  bank; start/stop only on the very first/last matmul of the bank.
"""

import numpy as np

P = 128
N_TOK = 4096
EMBED = 1024
NPAIR = 4            # head-pairs per core (8 heads)
ECH = EMBED // P     # 8 contraction chunks
XCH = 512            # projection chunk (tokens)
NCHP = N_TOK // XCH  # 8 projection chunks
TCH = N_TOK // P     # 32 token chunks of 128
NS_ITERS = 6

_CACHE = {}


def _build(global_scale=False, debug=False):
    del global_scale  # kept for test.py compat; no collective needed
    del debug
    import concourse.mybir as mybir
    from concourse import bacc
    from concourse.tile import TileContext
    from concourse.masks import make_identity

    f32 = mybir.dt.float32
    f32r = mybir.dt.float32r
    bf16 = mybir.dt.bfloat16
    ALU = mybir.AluOpType
    ACTF = mybir.ActivationFunctionType
    AX = mybir.AxisListType

    nc = bacc.Bacc("TRN2", target_bir_lowering=False, debug=False)
    X = nc.dram_tensor("X", [EMBED, N_TOK], bf16, kind="ExternalInput")
    WqT = nc.dram_tensor("WqT", [EMBED, 512], bf16, kind="ExternalInput")
    bias = nc.dram_tensor("bias", [512], f32, kind="ExternalInput")
    out_d = nc.dram_tensor("out", [N_TOK, 512], bf16, kind="ExternalOutput")

    with TileContext(nc) as tc, (
        tc.tile_pool(name="pers", bufs=1)
    ) as pers, tc.tile_pool(name="big", bufs=1) as big:
        # ---------------- persistent tiles ----------------
        ident32 = pers.tile([P, P], f32, tag="ident32")
        make_identity(nc, ident32[:])
        identb = pers.tile([P, P], bf16, tag="identb")
        nc.vector.tensor_copy(identb[:], ident32[:])
        identr = pers.tile([P, P], f32r, tag="identr")
        nc.vector.tensor_copy(identr[:], ident32[:])
        # packed a*I constants for 4-pair NS elementwise
        i7 = pers.tile([P, 512], f32, tag="i7")
        i15 = pers.tile([P, 512], f32, tag="i15")
        i13 = pers.tile([P, 512], f32, tag="i13")
        for t, v in ((i7, 7.0), (i15, 15.0), (i13, 13.0)):
            for hh in range(4):
                nc.vector.tensor_scalar_mul(t[:, hh * P:(hh + 1) * P], ident32[:], v)
        ones_col = pers.tile([P, 1], f32r, tag="ones_col")
        nc.vector.memset(ones_col[:].bitcast(f32), 1.0)
        ones_row = pers.tile([1, P], f32r, tag="ones_row")
        nc.vector.memset(ones_row[:].bitcast(f32), 1.0)
        bias_t = pers.tile([P, NPAIR], f32, tag="bias")
        nc.sync.dma_start(bias_t[:], bias.rearrange("(f p) -> p f", p=P))
        qsum = [pers.tile([P, 64], f32, tag=f"qsum{p}", name=f"qsum{p}")
                for p in range(NPAIR)]
        cp = [pers.tile([P, NCHP], f32, tag=f"cp{p}", name=f"cp{p}")
              for p in range(NPAIR)]
        qkvt = big.tile([P, NPAIR, N_TOK], bf16, tag="qkvt")
        st = big.tile([P, NPAIR, N_TOK], bf16, tag="st")
        qn = big.tile([P, TCH, 512], bf16, tag="qn")
        wqtr = pers.tile([P, ECH, 512], bf16, tag="wqtr")
        wqre = WqT.rearrange("(eo p) hd -> p eo hd", p=P)
        nc.sync.dma_start(wqtr[:, 0:4, :], wqre[:, 0:4, :])
        nc.sync.dma_start(wqtr[:, 4:8, :], wqre[:, 4:8, :])

        # ---------------- phase B: projection ----------------
        with (
            tc.tile_pool(name="x", bufs=2) as xpool,
            tc.tile_pool(name="pp", bufs=4, space="PSUM") as pp,
            tc.tile_pool(name="tq", bufs=2, space="PSUM") as tq,
        ):
            # PE warmup while the first DMAs land: keeps the p-state ramp hot
            wzero = pers.tile([P, 512], bf16, tag="wzero")
            nc.vector.memset(wzero[:], 0.0)
            for w in range(14):
                pw = pp.tile([P, 512], f32, tag="proj", name=f"warm{w}")
                nc.tensor.matmul(pw[:], identb[:], wzero[:], start=True, stop=True)

            xre = X.rearrange("(eo p) n -> p eo n", p=P)

            def qn_transpose(c):
                # qn: transpose QKV^T chunks -> [tok, hd]
                for sc in range(4):
                    t0 = c * 4 + sc
                    tsl = slice(t0 * P, (t0 + 1) * P)
                    psQ = tq.tile([P, 512], bf16, tag="psQ", name=f"psQ{t0}")
                    for p in range(NPAIR):
                        nc.tensor.matmul(
                            psQ[:, p * P:(p + 1) * P], qkvt[:, p, tsl], identb[:],
                            is_transpose=True, start=(p == 0), stop=(p == NPAIR - 1),
                            skip_group_check=True,
                        )
                    nc.vector.tensor_copy(qn[:, t0, :], psQ[:])

            for c in range(NCHP):
                xt = xpool.tile([P, ECH, XCH], bf16, tag="xt", name=f"xt{c}")
                nc.sync.dma_start(xt[:, 0:4, :], xre[:, 0:4, c * XCH:(c + 1) * XCH])
                nc.sync.dma_start(xt[:, 4:8, :], xre[:, 4:8, c * XCH:(c + 1) * XCH])
                csl = slice(c * XCH, (c + 1) * XCH)
                for p in range(NPAIR):
                    ps = pp.tile([P, XCH], f32, tag="proj", name=f"proj{c}_{p}")
                    for e in range(ECH):
                        nc.tensor.matmul(
                            ps[:],
                            wqtr[:, e, p * P:(p + 1) * P],
                            xt[:, e, :],
                            start=(e == 0),
                            stop=(e == ECH - 1),
                        )
                    # QKV^T (bf16) = psum + bias (per-partition)
                    nc.scalar.activation(
                        qkvt[:, p, csl], ps[:], ACTF.Identity,
                        bias=bias_t[:, p:p + 1],
                    )
                    # landmark window sums (64-token windows, pre-bias f32)
                    nc.vector.reduce_sum(
                        qsum[p][:, c * 8:(c + 1) * 8],
                        ps[:].rearrange("p (w t) -> p w t", t=64),
                        axis=AX.X,
                    )
                # transposes lag one chunk so PE never waits on Act's qkvt
                if c > 0:
                    qn_transpose(c - 1)
            qn_transpose(NCHP - 1)

        # ---------------- phases C-F ----------------
        with (
            tc.tile_pool(name="wk", bufs=4) as wk,
            tc.tile_pool(name="nsv", bufs=1) as nsp,
            tc.tile_pool(name="sn", bufs=4) as snp,
            tc.tile_pool(name="ot", bufs=6) as otp,
            tc.tile_pool(name="mps", bufs=1, space="PSUM") as mps,
        ):
            sctx = tc.tile_pool(name="stps", bufs=2, space="PSUM")
            stps = sctx.__enter__()
            cctx = tc.tile_pool(name="cps", bufs=1, space="PSUM")
            cps = cctx.__enter__()

            def st_chunk(j):
                jsl = slice(j * XCH, (j + 1) * XCH)
                for p in range(NPAIR):
                    psT = stps.tile([P, 512], f32, tag="psT", name=f"psT{j}_{p}")
                    nc.tensor.matmul(
                        psT[:], blkqb[p][:], qkvt[:, p, jsl], start=True, stop=True)
                    nc.scalar.activation(
                        st[:, p, jsl], psT[:], ACTF.Exp, scale=0.125,
                        accum_out=cp[p][:, j:j + 1],
                    )

            # ---- C: landmarks, Gamma, GD, NS init (per-head scale) ----
            blkq = []
            blkqb = []
            gd = [nsp.tile([P, P], f32r, tag=f"gd{p}", name=f"gd{p}")
                  for p in range(NPAIR)]
            kt0 = nsp.tile([P, 512], bf16, tag="kt", name="kt0")
            v00 = nsp.tile([P, 512], bf16, tag="v0", name="v00")
            vt00 = nsp.tile([P, 512], bf16, tag="vt", name="vt00")

            def c_chain(p):
                hsl = slice(p * P, (p + 1) * P)
                bq_t = pers.tile([P, P], f32r, tag=f"blkq{p}", name=f"blkq{p}")
                nc.vector.memset(bq_t[:].bitcast(f32), 0.0)
                nc.vector.tensor_scalar(
                    bq_t[0:64, 0:64], qsum[p][0:64, :], 1.0 / 64,
                    bias_t[0:64, p:p + 1], ALU.mult, ALU.add,
                )
                nc.vector.tensor_scalar(
                    bq_t[64:128, 64:128], qsum[p][64:128, :], 1.0 / 64,
                    bias_t[64:128, p:p + 1], ALU.mult, ALU.add,
                )
                blkq.append(bq_t)
                bq_b = pers.tile([P, P], bf16, tag=f"blkqb{p}", name=f"blkqb{p}")
                nc.gpsimd.tensor_copy(bq_b[:], bq_t[:])
                blkqb.append(bq_b)
                psG = cps.tile([P, P], f32, tag="psG", name=f"psG{p}")
                nc.tensor.matmul(psG[:], bq_t[:], bq_t[:], start=True, stop=True)
                gm = wk.tile([P, P], f32, tag="g", name=f"g{p}")
                nc.scalar.activation(gm[:], psG[:], ACTF.Exp, scale=0.125)
                nc.gpsimd.memset(gm[0:64, 64:128], 0.0)
                nc.gpsimd.memset(gm[64:128, 0:64], 0.0)
                gs = wk.tile([P, 1], f32, tag="gs", name=f"gs{p}")
                nc.vector.reduce_sum(gs[:], gm[:], axis=AX.X)
                gri = wk.tile([P, 1], f32, tag="gri", name=f"gri{p}")
                nc.vector.reciprocal(gri[:], gs[:])
                nc.vector.tensor_scalar_mul(gd[p][:], gm[:], gri[:])
                # per-head scale: rowsum(GD)=1 -> s = 1/max(colsum(GD)) per head
                psc = cps.tile([P, P], f32, tag="psc", name=f"psc{p}")
                nc.tensor.matmul(
                    psc[0:1, :], ones_col[:], gd[p][:], start=True, stop=True)
                cm = wk.tile([1, 2], f32, tag="cm", name=f"cm{p}")
                nc.vector.reduce_max(
                    cm[:], psc[0:1, :].rearrange("p (h l) -> p h l", l=64), axis=AX.X)
                cmi = wk.tile([1, 2], f32, tag="cmi", name=f"cmi{p}")
                nc.vector.reciprocal(cmi[:], cm[:])
                cmib = wk.tile([1, 2], f32r, tag="cmib", name=f"cmib{p}")
                nc.gpsimd.tensor_copy(cmib[:], cmi[:])
                psb = cps.tile([P, 2], f32, tag="psb", name=f"psb{p}")
                nc.tensor.matmul(psb[:], ones_row[:], cmib[:], start=True, stop=True)
                sv = wk.tile([P, 2], f32, tag="sv", name=f"sv{p}")
                nc.vector.tensor_copy(sv[:], psb[:])
                pskt = cps.tile([P, P], f32r, tag="pskt", name=f"pskt{p}")
                nc.tensor.matmul(pskt[:], gd[p][:], identr[:], is_transpose=True)
                nc.vector.tensor_copy(kt0[:, hsl], pskt[:])
                # v0 = s * K^T, vt0 = s * K  (s per head = per row-half)
                nc.vector.tensor_scalar_mul(
                    v00[0:64, hsl], pskt[0:64, :], sv[0:64, 0:1])
                nc.vector.tensor_scalar_mul(
                    v00[64:128, hsl], pskt[64:128, :], sv[64:128, 1:2])
                nc.gpsimd.tensor_scalar_mul(
                    vt00[0:64, hsl], gd[p][0:64, :], sv[0:64, 0:1])
                nc.gpsimd.tensor_scalar_mul(
                    vt00[64:128, hsl], gd[p][64:128, :], sv[64:128, 1:2])

            c_chain(0)
            c_chain(1)
            c_chain(2)
            c_chain(3)
            st_chunk(0)
            st_chunk(1)
            cctx.__exit__(None, None, None)
            mbank = mps.tile([P, NPAIR, P], f32, tag="mbank")

            # ---- D: interleaved NS iterations + ST + token loop ----
            dctx = tc.tile_pool(name="nsps", bufs=3, space="PSUM")
            nsps = dctx.__enter__()
            dctx3 = tc.tile_pool(name="tp2", bufs=2, space="PSUM")
            tp2 = dctx3.__enter__()
            quarts = [slice(h * P, (h + 1) * P) for h in range(4)]

            def ns_mm4(name, lhs, rhs):
                ps = nsps.tile([P, 512], f32, tag="nsb", name=name)
                for h, hs in enumerate(quarts):
                    nc.tensor.matmul(ps[:, hs], lhs[:, hs], rhs[:, hs],
                                     start=(h == 0), stop=(h == 3),
                                     skip_group_check=True)
                return ps

            ns_state = {"v": v00, "vt": vt00}

            def ns_step(it):
                v, vt = ns_state["v"], ns_state["vt"]
                pskv = ns_mm4(f"pskv{it}", kt0, v)
                pskvt = ns_mm4(f"pskvt{it}", v, kt0)
                a1 = nsp.tile([P, 512], bf16, tag="a1", name=f"a1_{it}")
                nc.vector.tensor_tensor(a1[:], i7[:], pskv[:], ALU.subtract)
                kvt = nsp.tile([P, 512], bf16, tag="kvt", name=f"kvt_{it}")
                nc.scalar.copy(kvt[:], pskvt[:])
                psa2 = ns_mm4(f"psa2_{it}", kvt, a1)
                a3 = nsp.tile([P, 512], bf16, tag="a3", name=f"a3_{it}")
                nc.vector.tensor_tensor(a3[:], i15[:], psa2[:], ALU.subtract)
                psa4 = ns_mm4(f"psa4_{it}", kvt, a3)
                a5 = nsp.tile([P, 512], bf16, tag="a5", name=f"a5_{it}")
                nc.vector.tensor_tensor(a5[:], i13[:], psa4[:], ALU.subtract)
                if it < NS_ITERS - 1:
                    psv = ns_mm4(f"psv_{it}", vt, a5)
                    vn = nsp.tile([P, 512], bf16, tag="v0", name=f"vn_{it}")
                    nc.vector.tensor_scalar_mul(vn[:], psv[:], 0.25)
                else:
                    vn = v
                psvt2 = ns_mm4(f"psvt2_{it}", a5, vt)
                vtn = nsp.tile([P, 512], bf16, tag="vt", name=f"vtn_{it}")
                nc.scalar.mul(vtn[:], psvt2[:], 0.25)
                ns_state.update(v=vn, vt=vtn)

            ns_per_j = [1, 1, 1, 1, 1, 1, 0, 0]
            ns_i = 0

            for j in range(NCHP):
                for _ in range(ns_per_j[j]):
                    ns_step(ns_i)
                    ns_i += 1
                if j >= 2:
                    st_chunk(j)
                for sc in range(4):
                    t0 = j * 4 + sc
                    tsl = slice(t0 * P, (t0 + 1) * P)
                    psS = tp2.tile([P, 512], bf16, tag="psS", name=f"psS{t0}")
                    for p in range(NPAIR):
                        nc.tensor.matmul(
                            psS[:, p * P:(p + 1) * P], st[:, p, tsl], identb[:],
                            is_transpose=True, start=(p == 0), stop=(p == NPAIR - 1),
                            skip_group_check=True,
                        )
                    sn = snp.tile([P, 512], bf16, tag="sn", name=f"sn{t0}")
                    nc.vector.tensor_copy(sn[:], psS[:])
                    for p in range(NPAIR):
                        nc.tensor.matmul(
                            mbank[:, p, :], sn[:, p * P:(p + 1) * P],
                            qn[:, t0, p * P:(p + 1) * P],
                            start=(t0 == 0 and p == 0),
                            stop=(t0 == TCH - 1 and p == NPAIR - 1),
                            skip_group_check=True,
                        )

            dctx3.__exit__(None, None, None)
            dctx.__exit__(None, None, None)
            sctx.__exit__(None, None, None)

            # ---- E: W = V6 @ (diag(1/c) M), plus ones cols for r ----
            ectx = tc.tile_pool(name="wps", bufs=1, space="PSUM")
            wps = ectx.__enter__()
            wpads = []
            for p in range(NPAIR):
                cs = wk.tile([P, 1], f32, tag="cs", name=f"cs{p}")
                nc.vector.reduce_sum(cs[:], cp[p][:], axis=AX.X)
                cinv = wk.tile([P, 1], f32, tag="cinv", name=f"cinv{p}")
                nc.vector.reciprocal(cinv[:], cs[:])
                dvp = wk.tile([P, P], bf16, tag="dvp", name=f"dvp{p}")
                nc.scalar.mul(dvp[:], mbank[:, p, :], cinv[:])
                # zero cross-head blocks (S/V are dense across the pair)
                nc.gpsimd.memset(dvp[0:64, 64:128], 0.0)
                nc.gpsimd.memset(dvp[64:128, 0:64], 0.0)
                psw = wps.tile([P, P], f32, tag="psw", name=f"psw{p}")
                vt6 = ns_state["vt"]
                nc.tensor.matmul(psw[:], vt6[:, p * P:(p + 1) * P], dvp[:],
                                 start=True, stop=True)
                wpad = pers.tile([P, 132], bf16, tag=f"wpad{p}")
                nc.gpsimd.memset(wpad[:], 0.0)
                nc.scalar.copy(wpad[:, 0:P], psw[:])
                nc.gpsimd.memset(wpad[0:64, 128:129], 1.0)
                nc.gpsimd.memset(wpad[64:128, 129:130], 1.0)
                wpads.append(wpad)

            # ---- F: out = diag(1/r) S W ----
            fctx = tc.tile_pool(name="fps", bufs=6, space="PSUM")
            fps = fctx.__enter__()
            unit = 0
            for t in range(TCH):
                tsl = slice(t * P, (t + 1) * P)
                if t % 2 == 0:
                    ot4 = otp.tile([P, 2, 512], bf16, tag="ot", name=f"ot{t}")
                ot = ot4[:, t % 2, :]
                for q in range(2):
                    pso = fps.tile([P, 260], f32, tag="pso", name=f"pso{q}_{t}")
                    for jj in range(2):
                        p = 2 * q + jj
                        nc.tensor.matmul(
                            pso[:, jj * 130:(jj + 1) * 130], st[:, p, tsl],
                            wpads[p][:, 0:130],
                            start=(jj == 0), stop=(jj == 1), skip_group_check=True,
                        )
                    rv4 = wk.tile([P, 2, 2, 1], f32, tag=f"rv{q}", name=f"rv{q}_{t}")
                    osl = ot[:, q * 256:(q + 1) * 256].rearrange(
                        "p (b h d) -> p b h d", h=2, d=64)
                    if unit % 2 == 0:
                        data = pso[:].rearrange("p (b x) -> p b x", b=2)
                        nc.vector.reciprocal(rv4[:, :, :, 0], data[:, :, 128:130])
                        nc.vector.tensor_tensor(
                            osl,
                            data[:, :, 0:128].rearrange("p b (h d) -> p b h d", d=64),
                            rv4[:].to_broadcast([P, 2, 2, 64]),
                            ALU.mult,
                        )
                    else:
                        pf = snp.tile([P, 260], f32, tag="pf", name=f"pf{t}_{q}")
                        nc.scalar.copy(pf[:], pso[:])
                        dataf = pf[:].rearrange("p (b x) -> p b x", b=2)
                        nc.vector.reciprocal(rv4[:, :, :, 0], dataf[:, :, 128:130])
                        nc.gpsimd.tensor_tensor(
                            osl,
                            dataf[:, :, 0:128].rearrange("p b (h d) -> p b h d", d=64),
                            rv4[:].to_broadcast([P, 2, 2, 64]),
                            ALU.mult,
                        )
                    unit += 1
                if t % 2 == 1:
                    t4sl = slice((t - 1) * P, (t + 1) * P)
                    nc.sync.dma_start(
                        out_d[t4sl, :].rearrange("(c p) n -> p c n", p=P), ot4[:])
            fctx.__exit__(None, None, None)
            ectx.__exit__(None, None, None)

    nc.compile()
    return nc


def _get_nc():
    if "nc" not in _CACHE:
        _CACHE["nc"] = _build()
    return _CACHE["nc"]


def kernel(X, Wq, bq):
    import ml_dtypes
    from concourse.bass_utils import run_bass_kernel_spmd

    nc = _get_nc()
    B, E, n = X.shape
    H = Wq.shape[0]
    in_maps = []
    for core in range(8):
        b = core // 2
        h0 = 8 * (core % 2)
        wq_c = Wq[h0:h0 + 8]                      # [8, 64, 1024]
        wqt_c = np.ascontiguousarray(
            wq_c.transpose(2, 0, 1).reshape(E, 512)).astype(ml_dtypes.bfloat16)
        bias_c = np.ascontiguousarray(bq[h0:h0 + 8].reshape(512))
        in_maps.append({
            "X": np.ascontiguousarray(X[b]).astype(ml_dtypes.bfloat16),
            "WqT": wqt_c,
            "bias": bias_c,
        })
    res = run_bass_kernel_spmd(nc, in_maps, core_ids=list(range(8)))
    out = np.empty((B, H, n, 64), dtype=np.float32)
    for core in range(8):
        b = core // 2
        h0 = 8 * (core % 2)
        oc = np.asarray(res.results[core]["out"]).astype(np.float32).reshape(n, 8, 64)
        out[b, h0:h0 + 8] = oc.transpose(1, 0, 2)
    return out
